# revision 13
# baseline (speedup 1.0000x reference)
"""Trainium2 Bass kernel for LocalSingularityStrength (multi-scale box-filter
OLS slope + BN inference), data-parallel over 8 NeuronCores.

Reference per sample (H=224, W=224, C=32):
  1. xs = (x - mn) / (mx - mn + 1e-7)                      (per-sample minmax)
  2. m_r = 2D box sum of xs with SAME padding, r in {2,4,8,16}
  3. alpha = OLS slope of ln(m_r + 1e-7) vs ln(r)
  4. out = (alpha - mu) / sqrt(var + 1e-3) * gamma + beta

Key algebraic restructuring used here: with rng = mx - mn + eps,
  ln(m_r + eps) = ln(S_r - mn*A_r + eps*rng) - ln(rng)
where S_r is the box sum of RAW x (zero padded) and A_r(h,w) = ah_r(h)*aw_r(w)
is the (separable) valid-pixel count.  The -ln(rng) term is constant across
scales so it cancels in the OLS slope.  Therefore:
  - no per-pixel normalization pass at all;
  - the -mn*A_r term is added by the H-direction matmul via one extra
    contraction row: lhsT row K holds -mn*ah_r(m) (runtime, tiny update),
    rhs row K holds aw_r(w) (compile-time constant);
  - eps*rng rides the Ln activation bias (per-partition scalar).

Mapping per core (BS = 2 samples):
  - batch sharded 2 samples/core across 8 cores (pure data parallel).
  - per sample, 2 row-jobs of M=112 output rows, each loading K=120 input
    rows (job0: 0..119, job1: 104..223) so K is uniform.
  - W-direction box sums: doubling shift-add cascade on VectorE in fp16,
    computed per 112-column stripe into double-buffered tiles so the PE
    matmuls of stripe s overlap the cascade of stripe s+1.
  - H-direction box sums + rank-1 minmax correction: banded matmuls on
    TensorE (contraction K+1 = 121 <= 128), fp32 PSUM.
  - ln(m + eps*rng): ScalarE activation out of PSUM (evacuation fused).
  - OLS combine: merged pair-subtract on VectorE (one 2x fp16 op/stripe),
    then (d0/3 + d1) and BN affine on the (otherwise idle) Pool engine.
  - output written fp16 to HBM; host upcasts to fp32.
"""

import math
import sys

sys.path.insert(0, "/opt/trn_rl_repo")

import numpy as np

import concourse.bacc as bacc
import concourse.bass as bass
import concourse.tile as tile
from concourse import mybir
from concourse.bass_utils import run_bass_kernel_spmd

FP16 = mybir.dt.float16
FP32 = mybir.dt.float32
ALU = mybir.AluOpType
ACT = mybir.ActivationFunctionType

NCORES = 8
SCALES = [2, 4, 8, 16]
NS = len(SCALES)
# lc scale slot order: [4, 2, 8, 16] so that d = lc[:,2:4,:] - lc[:,0:2,:]
# gives (L8-L4, L16-L2) with positive strides everywhere.
LC_ORDER = [4, 2, 8, 16]
EPS_K = 1e-7
BN_EPS = 1e-3
PAD_L = 7
PAD_R = 9  # WP = W + 16
KROWS = 120  # uniform input rows per job
STRIPE_W = 112  # output w columns per stripe
STRIPE_HALO = 7  # input reach left
STRIPE_IN_W = 128  # 7 + 112 + 9 input columns per stripe tile

# OLS weights: alpha = sum_s q_s * ln(m_s); with the (L8-L4)/3 + (L16-L2)
# pairing, alpha = Q16 * ((L8-L4)/3 + (L16-L2)).
_ls = np.log(np.array(SCALES, dtype=np.float64))
_dls = _ls - _ls.mean()
_den = float((_dls**2).sum())
Q16 = float(_dls[3] / _den)


def _jobs(H):
    """(out_start, out_end, in_start) with uniform K=KROWS input rows."""
    M = H // 2
    return [(0, M, 0), (M, H, H - KROWS)]


def _win(r):
    lo = (r - 1) // 2
    hi = r // 2
    return lo, hi


def _make_consts(H, W, C):
    """Host-side constant tensors.

    bands:  [2*NS, 128, M] fp16; rows 0..K-1 banded ones, row K zero (dynamic)
    ahc:    [2*NS, 128, M] fp16; row K = ah_r(h) valid-row count, else zero
    awrow:  [NS, 2, SIN*C] fp16; aw_r(w) per stripe parity, replicated over C
    """
    M = H // 2
    jobs = _jobs(H)
    SIN = STRIPE_IN_W
    bands = np.zeros((2 * NS, 128, M), np.float16)
    ahc = np.zeros((2 * NS, M), np.float16)
    for jt, (a, b, lo_in) in enumerate(jobs):
        for si, r in enumerate(SCALES):
            lo, hi = _win(r)
            blk = jt * NS + si
            for m in range(M):
                h = a + m
                r0 = max(0, h - lo)
                r1 = min(H - 1, h + hi)
                bands[blk, r0 - lo_in : r1 - lo_in + 1, m] = 1.0
                ahc[blk, m] = r1 - r0 + 1
    nstripes = W // STRIPE_W
    assert nstripes == 2, "stripe parity trick assumes W == 2*STRIPE_W"
    awrow = np.zeros((NS, 2, SIN * C), np.float16)
    for si, r in enumerate(SCALES):
        lo, hi = _win(r)
        for sp in range(2):
            for wi in range(SIN):
                w = sp * STRIPE_W - STRIPE_HALO + wi
                if 0 <= w < W:
                    aw = min(W - 1, w + hi) - max(0, w - lo) + 1
                    awrow[si, sp, wi * C : (wi + 1) * C] = aw
    return bands, ahc, awrow


def build_program(BS, H, W, C, n_cores=NCORES):
    assert H % 2 == 0
    M = H // 2
    assert M + 16 <= 128 and KROWS + 1 <= 128
    WP = W + PAD_L + PAD_R
    SIN = STRIPE_IN_W
    NST = W // STRIPE_W  # stripes per job (2)
    CPS = STRIPE_W * C // 512  # 512-chunks per stripe (7)
    K = KROWS
    jobs = _jobs(H)

    uniform, scq_imm, bi_imm = _BN_MODE

    nc = bacc.Bacc("TRN2", target_bir_lowering=False, debug=False, num_devices=n_cores)
    x_in = nc.dram_tensor("x", [BS, H, W, C], FP32, kind="ExternalInput")
    bands_in = nc.dram_tensor("bands", [2 * NS, 128, M], FP16, kind="ExternalInput")
    ahc_in = nc.dram_tensor("ahc", [2 * NS, M], FP16, kind="ExternalInput")
    awrow_in = nc.dram_tensor("awrow", [NS, 2, SIN * C], FP16, kind="ExternalInput")
    scq_in = nc.dram_tensor("scq", [C], FP32, kind="ExternalInput")
    bi_in = nc.dram_tensor("bi", [C], FP32, kind="ExternalInput")
    out_t = nc.dram_tensor("out", [BS, H, W, C], FP16, kind="ExternalOutput")

    with tile.TileContext(nc) as tc:
        with (
            tc.tile_pool(name="consts", bufs=1) as consts,
            tc.tile_pool(name="xraw", bufs=1) as xraw_pool,
            tc.tile_pool(name="wts", bufs=1) as wts_pool,
            tc.tile_pool(name="small", bufs=4) as small,
            tc.tile_pool(name="lcp", bufs=1) as lcp,
            tc.tile_pool(name="dp", bufs=1) as dp,
            tc.tile_pool(name="up", bufs=1) as up,
            tc.tile_pool(name="ocp", bufs=1) as ocp,
            tc.tile_pool(name="psum", bufs=2, space="PSUM") as psum_pool,
        ):
            # ---- constants ----
            band_sb = consts.tile([128, 2 * NS, M], FP16)
            nc.sync.dma_start(out=band_sb, in_=bands_in.rearrange("s k m -> k s m"))
            # ah row-count constants live on partition 0; the dynamic
            # -mn*ah row is computed there and DMA'd into band row K
            # (compute engines cannot address a lone partition 120).
            ahc_sb = consts.tile([1, 2 * NS, M], FP16)
            nc.sync.dma_start(out=ahc_sb, in_=ahc_in[0 : 2 * NS])
            if not uniform:
                scq_sb = consts.tile([128, C], FP32)
                bi_sb = consts.tile([128, C], FP32)
                for dst, src in ((scq_sb, scq_in), (bi_sb, bi_in)):
                    nc.sync.dma_start(
                        out=dst,
                        in_=bass.AP(tensor=src.tensor, offset=0, ap=[[0, 128], [1, C]]),
                    )

            # ---- persistent tiles ----
            xraw = [
                xraw_pool.tile([128, WP * C], FP32, name=f"xraw{i}", tag=f"xraw{i}")
                for i in range(2)
            ]
            # zero the pads once (DMA writes only the center)
            for i in range(2):
                nc.vector.memset(xraw[i][:, 0 : PAD_L * C], 0.0)
                nc.vector.memset(xraw[i][:, (PAD_L + W) * C : WP * C], 0.0)

            # striped, double-buffered cascade tiles; wt[parity][r]
            wt = [
                {
                    r: wts_pool.tile(
                        [128, SIN * C], FP16, name=f"w{p}_{r}", tag=f"w{p}_{r}"
                    )
                    for r in SCALES
                }
                for p in range(2)
            ]
            # row K of each wt tile = aw_r for that stripe parity (constant)
            for p in range(2):
                for si, r in enumerate(SCALES):
                    nc.sync.dma_start(
                        out=wt[p][r][K : K + 1, :],
                        in_=awrow_in[si, p : p + 1],
                    )

            for b in range(BS):
                # ---- load both jobs ----
                for j, (a0, b0, lo_in) in enumerate(jobs):
                    nc.sync.dma_start(
                        out=xraw[j][0:K, PAD_L * C : (PAD_L + W) * C],
                        in_=x_in[b, lo_in : lo_in + K].rearrange("k w c -> k (w c)"),
                    )

                # ---- per-sample min / max ----
                # column layout: [max_j0, max_j1, min_j0, min_j1]
                rr = small.tile([128, 4], FP32, tag="rr")
                for j in range(2):
                    xc = xraw[j][0:K, PAD_L * C : (PAD_L + W) * C]
                    nc.vector.tensor_reduce(
                        out=rr[0:K, j : j + 1],
                        in_=xc,
                        axis=mybir.AxisListType.X,
                        op=ALU.max,
                    )
                    nc.vector.tensor_reduce(
                        out=rr[0:K, 2 + j : 3 + j],
                        in_=xc,
                        axis=mybir.AxisListType.X,
                        op=ALU.min,
                    )
                # transpose [K,4] -> [1,K,4] via SBUF->SBUF DMA (rows >= K are
                # never written and must stay out of the final reduce)
                rf = small.tile([1, K, 4], FP32, tag="rf")
                nc.sync.dma_start(out=rf, in_=rr[0:K, :])
                mx = small.tile([1, 1], FP32, tag="mx")
                mnv = small.tile([1, 1], FP32, tag="mnv")
                nc.vector.tensor_reduce(
                    out=mx, in_=rf[:, :, 0:2], axis=mybir.AxisListType.XY, op=ALU.max
                )
                nc.vector.tensor_reduce(
                    out=mnv, in_=rf[:, :, 2:4], axis=mybir.AxisListType.XY, op=ALU.min
                )
                nmn = small.tile([1, 1], FP32, tag="nmn")
                nc.vector.tensor_scalar(
                    out=nmn, in0=mnv, scalar1=-1.0, scalar2=None, op0=ALU.mult
                )
                rng = small.tile([1, 1], FP32, tag="rng")
                nc.vector.tensor_tensor(out=rng, in0=mx, in1=nmn, op=ALU.add)
                epsr = small.tile([1, 1], FP32, tag="epsr")
                # eps*(rng + eps) = eps*rng'' with rng'' = mx-mn+eps
                nc.vector.tensor_scalar(
                    out=epsr, in0=rng, scalar1=EPS_K, scalar2=EPS_K, op0=ALU.add, op1=ALU.mult
                )
                # Ln bias needs eps*rng on partitions 0..M-1
                epsrb = small.tile([128, 1], FP32, tag="epsrb")
                nc.gpsimd.partition_broadcast(epsrb, epsr, channels=128)
                # dynamic band rows: band_sb[K, blk, m] = -mn * ah_r(m),
                # computed on partition 0 then DMA'd into partition K
                dyn = small.tile([1, 2 * NS, M], FP16, tag="dyn")
                nc.vector.tensor_scalar(
                    out=dyn,
                    in0=ahc_sb,
                    scalar1=nmn[0:1],
                    scalar2=None,
                    op0=ALU.mult,
                )
                nc.sync.dma_start(out=band_sb[K : K + 1, :, :], in_=dyn)

                # ---- jobs x stripes ----
                for j, (a0, b0, lo_in) in enumerate(jobs):
                    for sp in range(NST):
                        w0 = sp * STRIPE_W  # output stripe start
                        # absolute input col range [w0-7, w0+121) in padded coords
                        xb = (w0 - STRIPE_HALO + PAD_L) * C
                        wtp = wt[sp]

                        # cascade (relative widths shrink per level)
                        def sadd(dst, dw0, dw1, src, s0, s1, src_base):
                            nc.vector.tensor_tensor(
                                out=dst[0:K, dw0 * C : dw1 * C],
                                in0=src[0:K, src_base + (dw0 + s0) * C : src_base + (dw1 + s0) * C],
                                in1=src[0:K, src_base + (dw0 + s1) * C : src_base + (dw1 + s1) * C],
                                op=ALU.add,
                            )

                        # wt2[u] = x[u] + x[u+1]        u in [0,126)
                        sadd(wtp[2], 0, SIN - 2, xraw[j], 0, 1, xb)
                        # wt4[u] = wt2[u-1] + wt2[u+1]  u in [1,125)
                        sadd(wtp[4], 1, SIN - 3, wtp[4 // 2], -1, 1, 0)
                        # wt8[u] = wt4[u-2] + wt4[u+2]  u in [3,123)
                        sadd(wtp[8], 3, SIN - 5, wtp[4], -2, 2, 0)
                        # wt16[u] = wt8[u-4] + wt8[u+4] u in [7,119)
                        sadd(wtp[16], 7, SIN - 9, wtp[8], -4, 4, 0)

                        lc = lcp.tile([M, NS, CPS * 512], FP16, tag="lc")
                        for ci in range(CPS):
                            # chunk covers output w [w0+16ci, w0+16ci+16);
                            # relative to stripe tile: u = 7 + 16*ci
                            ub = (STRIPE_HALO + 16 * ci) * C
                            ps = psum_pool.tile([M, NS, 512], FP32)
                            for li, r in enumerate(LC_ORDER):
                                nc.tensor.matmul(
                                    ps[:, li, :],
                                    lhsT=band_sb[0 : K + 1, j * NS + SCALES.index(r), :],
                                    rhs=wtp[r][0 : K + 1, ub : ub + 512],
                                    start=True,
                                    stop=True,
                                )
                            nc.scalar.activation(
                                out=lc[:, :, ci * 512 : (ci + 1) * 512],
                                in_=ps,
                                func=ACT.Ln,
                                bias=epsrb[0:M],
                                scale=1.0,
                            )
                        # d = (L8 - L4, L16 - L2)
                        d = dp.tile([M, 2, CPS * 512], FP16, tag="d")
                        nc.vector.tensor_tensor(
                            out=d, in0=lc[:, 2:4, :], in1=lc[:, 0:2, :], op=ALU.subtract
                        )
                        oc = ocp.tile([M, CPS * 512], FP16, tag="oc")
                        if uniform:
                            # oc = (d0/3 + d1)*scq + bi, split as two Pool
                            # tensor_scalars + one DVE fp16 add (Pool has no
                            # scalar_tensor_tensor)
                            t1 = up.tile([M, CPS * 512], FP16, tag="t1")
                            t2 = up.tile([M, CPS * 512], FP16, tag="t2")
                            nc.gpsimd.tensor_scalar(
                                out=t1,
                                in0=d[:, 0, :],
                                scalar1=scq_imm / 3.0,
                                scalar2=bi_imm,
                                op0=ALU.mult,
                                op1=ALU.add,
                            )
                            nc.gpsimd.tensor_scalar(
                                out=t2,
                                in0=d[:, 1, :],
                                scalar1=scq_imm,
                                scalar2=None,
                                op0=ALU.mult,
                            )
                            nc.vector.tensor_tensor(out=oc, in0=t1, in1=t2, op=ALU.add)
                        else:
                            u = up.tile([M, CPS * 512], FP16, tag="u")
                            nc.vector.scalar_tensor_tensor(
                                out=u,
                                in0=d[:, 0, :],
                                scalar=1.0 / 3.0,
                                in1=d[:, 1, :],
                                op0=ALU.mult,
                                op1=ALU.add,
                            )
                            scq_ap = bass.AP(
                                tensor=scq_sb.tensor, offset=scq_sb.offset,
                                ap=[scq_sb.ap[0][:], [0, STRIPE_W], [1, C]],
                            )
                            bi_ap = bass.AP(
                                tensor=bi_sb.tensor, offset=bi_sb.offset,
                                ap=[bi_sb.ap[0][:], [0, STRIPE_W], [1, C]],
                            )
                            m1 = up.tile([M, CPS * 512], FP32, tag="m1")
                            nc.vector.tensor_tensor(
                                out=m1.rearrange("p (w c) -> p w c", c=C),
                                in0=u.rearrange("p (w c) -> p w c", c=C),
                                in1=scq_ap[0:M],
                                op=ALU.mult,
                            )
                            nc.vector.tensor_tensor(
                                out=oc.rearrange("p (w c) -> p w c", c=C),
                                in0=m1.rearrange("p (w c) -> p w c", c=C),
                                in1=bi_ap[0:M],
                                op=ALU.add,
                            )
                        nc.sync.dma_start(
                            out=out_t[b, a0:b0, w0 : w0 + STRIPE_W, :].rearrange(
                                "m w c -> m (w c)"
                            ),
                            in_=oc,
                        )

    nc.compile()
    return nc


# (uniform, scq_imm, bi_imm) — set by kernel() before build; default uniform
_BN_MODE = (True, Q16, 0.0)

_PROG_CACHE = {}


def _get_program(BS, H, W, C, bn_mode):
    key = (BS, H, W, C, bn_mode)
    if key not in _PROG_CACHE:
        global _BN_MODE
        _BN_MODE = bn_mode
        _PROG_CACHE[key] = build_program(BS, H, W, C)
    return _PROG_CACHE[key]


def _bn_fold(gamma, beta, moving_mean, moving_var):
    sc = gamma / np.sqrt(moving_var + np.float32(BN_EPS))
    scq = (sc * np.float32(Q16)).astype(np.float32)
    bi = (beta - moving_mean * sc).astype(np.float32)
    uniform = bool(np.ptp(scq) == 0 and np.ptp(bi) == 0)
    bn_mode = (uniform, float(scq[0]), float(bi[0])) if uniform else (False, 0.0, 0.0)
    return scq, bi, bn_mode


def _build_in_maps(x, gamma, beta, moving_mean, moving_var):
    B, H, W, C = x.shape
    BS = B // NCORES
    scq, bi, bn_mode = _bn_fold(gamma, beta, moving_mean, moving_var)
    bands, ahc, awrow = _make_consts(H, W, C)
    x_np = np.ascontiguousarray(x, dtype=np.float32)
    in_maps = []
    for i in range(NCORES):
        in_maps.append(
            {
                "x": x_np[i * BS : (i + 1) * BS],
                "bands": bands,
                "ahc": ahc,
                "awrow": awrow,
                "scq": scq,
                "bi": bi,
            }
        )
    return in_maps, bn_mode


def kernel(x, gamma, beta, moving_mean, moving_var):
    x = np.asarray(x)
    gamma = np.asarray(gamma, dtype=np.float32)
    beta = np.asarray(beta, dtype=np.float32)
    moving_mean = np.asarray(moving_mean, dtype=np.float32)
    moving_var = np.asarray(moving_var, dtype=np.float32)

    B, H, W, C = x.shape
    assert B % NCORES == 0
    BS = B // NCORES

    in_maps, bn_mode = _build_in_maps(x, gamma, beta, moving_mean, moving_var)
    nc = _get_program(BS, H, W, C, bn_mode)
    res = run_bass_kernel_spmd(nc, in_maps, list(range(NCORES)))
    out = np.concatenate([res.results[i]["out"] for i in range(NCORES)], axis=0)
    return out.astype(np.float32)
